# revision 97
# baseline (speedup 1.0000x reference)
"""Bass/Trainium2 kernel for batched 3D FFT circular convolution.

Reference computes: y = Re(IFFT3(FFT3(x) . FFT3(w))) with 1/sqrt(N) net
scaling, x: (16, 32, 128, 128) f32, w: (32, 128, 128) f32.

Strategy (pure data parallel over batch, 8 cores x 2 samples):
- Pack two real samples as one complex volume z = x0 + i*x1. Then
  y_pair = IFFT3(FFT3(z) * W~) and y0 = Re, y1 = Im (exact because w real).
- FFTs as DFT-matrix matmuls on the tensor engine, all in bf16 (the
  harness gate is 2e-2 L2; the all-bf16 chain measures ~7e-3). Axis
  rotations (partition<->free transposes) are FUSED into the FFT matmuls
  by making the DATA block the stationary operand and a concatenated
  [G_R | G_I] (128x256) DFT matrix the moving operand: out =
  Z_blk^T @ [G_R | G_I] yields the transposed, transformed block at full
  rate. Complex combine via psum accumulation with a second matmul
  against [-G_I | G_R]. No standalone PE transposes remain.
- The size-32 axis (d1) uses a block-diagonal 4x(32x32) DFT so the full
  128-partition contraction stays busy; its fwd stage and the final
  inverse d2 stage are classic F-stationary matmuls.
- W~ = FFT3(w)*alpha is computed on-device per core, exploiting w's
  REALNESS (Hermitian spectrum): W1 emits only k2l = k2%32 < 17 (a
  68-column trimmed moving operand), W2/W3 process only those 17 of 32
  k2l-blocks, and blocks k2l' in 17..31 are reconstructed as
  conj(W~(-k1,-k2,-k3)) of block 32-k2l': the free-axis reversal runs on
  an ALU copy (negative free strides), the partition permutation goes
  through a DRAM scratch + gpsimd indirect row-gather DMA (the only
  partition-permuting path whose descriptors stay whole contiguous
  rows), and the conjugation folds into the pointwise sign variant
  (chunks 2-3 use V_R = t1+t2, V_I = t4-t3; W3's block 16 stores a
  negated imaginary part so chunk 2 is sign-uniform). This cuts the
  W-chain PE time roughly in half.
- The pointwise z*W~ stages psum halves to bf16 SBUF (ACT), multiplies
  on DVE (bf16 2x mode) and Pool, per 512-col half so S4's groups
  unlock early.
- Inputs/outputs ride in DRAM pre-transposed to (D2, D1, D3) bf16 for
  full-rate contiguous DMA runs, spread over the SP/ACT/Pool(SWDGE)
  queues. PSUM evictions rotate over ACT/DVE (GPSIMD cannot touch PSUM
  on TRN2).

Layouts per stage (partition | free):
  load [d2 | d1,d3]
  W1/S1 ds-fwd d2 -> [d3 | k2l,(k2h,d1)]     (group-interleaved; scatter
                                              eviction so S2's stationary
                                              is contiguous)
  W2/S2 ds-fwd d3 -> [(k2h,d1) | k2l,k3]
  W3/S3 Fstat BD fwd -> [(k2h,k1) | k2l,k3]; S3 fuses * W~ pointwise
  S4 ds-inv k1    -> [k3 | d1',k2]           (scatter eviction)
  S5 ds-inv k3    -> [k2 | d1',d3']
  S6 Fstat inv k2 -> [d2' | d1',d3'] -> DMA out (chunked, 3 queues)
"""

import numpy as np

D1, D2, D3 = 32, 128, 128
NTOT = D1 * D2 * D3
FREE = D1 * D3  # 4096
B = 16
NCORES = 8

# w's spectrum is Hermitian (w real): the W chain computes only k2l =
# k2 mod 32 in 0..LW-1 (17 of 32 k2l-blocks) through W1/W2/W3; blocks
# k2l' in 17..31 of W~ are reconstructed from block 32-k2l' by a DRAM
# round-trip DMA that reverses k1/k2h/k3 (the conjugation is folded into
# the pointwise stage's sign variant).
LW = 17

# bf16 const column offsets in the packed (128, CB_TOT) consts input.
# ds pairs are A = [G_R|G_I] (256 cols), B = [-G_I|G_R] (256 cols).
CB_WS = 0               # A of fwd 128-DFT scaled by alpha (W1, real input),
#                         trimmed to k2l < LW: [GR_sel | GI_sel], 2*4*LW cols
CB_AF, CB_BF = 8 * LW, 8 * LW + 256     # fwd 128-DFT ds pair (S1/S2/W2)
CB_EARLY = 8 * LW + 512  # cols needed by W1/S1; the rest loads later
CB_ABI, CB_BBI = CB_EARLY, CB_EARLY + 256    # inv block-diag 32-DFT (S4)
CB_AFI, CB_BFI = CB_EARLY + 512, CB_EARLY + 768   # inv 128-DFT ds pair (S5)
CB_F2R, CB_F2I, CB_F2In = (CB_EARLY + 1024, CB_EARLY + 1152,
                           CB_EARLY + 1280)  # fstat inv k2 (S6)
CB_BDR, CB_BDI, CB_BDIn = (CB_EARLY + 1408, CB_EARLY + 1536,
                           CB_EARLY + 1664)  # fstat fwd block-diag (S3/W3)
CB_TOT = CB_EARLY + 1792


def _consts_bf16_np():
    """All twiddle matrices, packed bf16: (128, CB_TOT)."""
    import ml_dtypes

    k = np.arange(128)
    F2 = np.exp(-2j * np.pi * np.outer(k, k) / 128)
    k1 = np.arange(32)
    F1 = np.exp(-2j * np.pi * np.outer(k1, k1) / 32)
    BD = np.zeros((128, 128), complex)
    for g in range(4):
        BD[32 * g:32 * g + 32, 32 * g:32 * g + 32] = F1
    alpha = 1.0 / (NTOT * np.sqrt(np.float32(NTOT), dtype=np.float64))

    def ds_pair(G):
        A = np.concatenate([G.real, G.imag], axis=1)      # [128, 256]
        Bm = np.concatenate([-G.imag, G.real], axis=1)    # [128, 256]
        return A, Bm

    A_f, B_f = ds_pair(F2)
    A_fi, B_fi = ds_pair(np.conj(F2))
    A_bi, B_bi = ds_pair(np.conj(BD))

    # W1 trimmed moving operand: columns k2 = 32g + l, l < LW, g-major,
    # [GR_sel | GI_sel], scaled by alpha
    Fs = F2 * alpha
    sel = np.concatenate([np.arange(32 * g, 32 * g + LW) for g in range(4)])
    A_ws = np.concatenate([Fs.real[:, sel], Fs.imag[:, sel]], axis=1)

    m = np.zeros((128, CB_TOT))
    m[:, CB_WS:CB_WS + 8 * LW] = A_ws
    for c0, mat in ((CB_AF, A_f), (CB_BF, B_f),
                    (CB_ABI, A_bi), (CB_BBI, B_bi),
                    (CB_AFI, A_fi), (CB_BFI, B_fi)):
        m[:, c0:c0 + 256] = mat
    for c0, mat in ((CB_F2R, F2.real), (CB_F2I, F2.imag),
                    (CB_F2In, -F2.imag), (CB_BDR, BD.real),
                    (CB_BDI, BD.imag), (CB_BDIn, -BD.imag)):
        m[:, c0:c0 + 128] = mat
    return np.ascontiguousarray(m).astype(ml_dtypes.bfloat16)


def _ridx_np():
    """Row-gather indices for the Hermitian reconstruction: partition p' of
    the reconstructed region reads scratch row 32*(3-k2h') + (32-k1')%32
    of its plane."""
    p = np.arange(128)
    k2h, k1 = p // 32, p % 32
    pi0 = 32 * (3 - k2h) + (32 - k1) % 32
    return np.ascontiguousarray(
        np.stack([pi0, pi0], axis=1).astype(np.int32))


_STAGE_MARKS = []


def _mark(nc, label):
    try:
        n = int(nc.get_next_instruction_name().split("-")[1])
    except Exception:
        n = -1
    _STAGE_MARKS.append((n, label))


def _build_program():
    import concourse.mybir as mybir
    import concourse.tile as tile
    from concourse import bacc
    from concourse import bass

    f32 = mybir.dt.float32
    f32r = mybir.dt.float32r
    bf16 = mybir.dt.bfloat16

    nc = bacc.Bacc("TRN2")
    # x/w/y live in DRAM pre-transposed to (D2, D1, D3) so each partition
    # (= d2) reads/writes one long contiguous run (>=2KB per chunk) at full
    # DMA rate; bf16 halves the bytes (error budget: harness gate is 2e-2).
    x0_d = nc.dram_tensor("x0", (D2, D1, D3), bf16, kind="ExternalInput")
    x1_d = nc.dram_tensor("x1", (D2, D1, D3), bf16, kind="ExternalInput")
    w_d = nc.dram_tensor("w", (D2, D1, D3), bf16, kind="ExternalInput")
    cb_d = nc.dram_tensor("constsb", (128, CB_TOT), bf16,
                          kind="ExternalInput")
    ridx_d = nc.dram_tensor("ridx", (128, 2), mybir.dt.int32,
                            kind="ExternalInput")
    y0_d = nc.dram_tensor("y0", (D2, D1, D3), bf16, kind="ExternalOutput")
    y1_d = nc.dram_tensor("y1", (D2, D1, D3), bf16, kind="ExternalOutput")
    # DRAM scratch for the Hermitian W~ reconstruction (row r holds the
    # free-reversed source REVF[r], so reads only permute whole rows).
    # One tensor per plane: a shared tensor would make each whole-tensor
    # indirect read falsely depend on the other plane's write.
    wscR_d = nc.dram_tensor("wscR", (128, 15 * 128), bf16,
                            kind="ExternalOutput")
    wscI_d = nc.dram_tensor("wscI", (128, 15 * 128), bf16,
                            kind="ExternalOutput")

    with tile.TileContext(nc) as tc:
        with (
            tc.tile_pool(name="sb", bufs=1) as sb,
            tc.tile_pool(name="tp", bufs=2) as tp,
            tc.tile_pool(name="ps", bufs=4, space="PSUM") as ps,
        ):
            cb = sb.tile([128, CB_TOT], bf16, name="cb")

            def M(c0):
                return cb[:, c0:c0 + 128]

            def M2(c0):
                return cb[:, c0:c0 + 256]

            zR = [sb.tile([128, FREE], bf16, name=f"zR{c}") for c in range(2)]
            zI = [sb.tile([128, FREE], bf16, name=f"zI{c}") for c in range(2)]
            wR = [sb.tile([128, FREE], bf16, name=f"wR{c}") for c in range(2)]
            wI = [sb.tile([128, FREE], bf16, name=f"wI{c}") for c in range(2)]
            xR = sb.tile([128, FREE], bf16, name="xR")
            xI = sb.tile([128, FREE], bf16, name="xI")
            wL = sb.tile([128, FREE], bf16, name="wL")
            yR = sb.tile([128, FREE], bf16, name="yR")
            yI = sb.tile([128, FREE], bf16, name="yI")

            # input DMAs: partition = d2, contiguous (d1,d3) runs
            def load3(dst, src_d, a0, a1, eng=None):
                (eng or nc.sync).dma_start(
                    out=dst.rearrange("p (a c) -> p a c", a=D1)[:, a0:a1],
                    in_=src_d.ap()[:, a0:a1])

            # three parallel input queues: w and x1 interleaved on Pool
            # (SWDGE) so W1's stationaries and S1's imaginary-side operands
            # both arrive just ahead of their group-interleaved matmuls;
            # the early consts on ACT; x0 on SP. W1 starts ~2.5us in.
            load3(wL, w_d, 0, 4)
            load3(wL, w_d, 4, 12, eng=nc.gpsimd)
            load3(wL, w_d, 12, 20, eng=nc.gpsimd)
            load3(wL, w_d, 20, 32, eng=nc.gpsimd)
            for a0 in range(0, 32, 8):
                load3(xI, x1_d, a0, a0 + 8, eng=nc.gpsimd)
            nc.scalar.dma_start(out=cb[:, 0:CB_EARLY],
                                in_=cb_d.ap()[:, 0:CB_EARLY])
            ridx = sb.tile([128, 2], mybir.dt.int32, name="ridx")
            nc.scalar.dma_start(out=ridx, in_=ridx_d.ap())
            load3(xR, x0_d, 0, 8)
            nc.sync.dma_start(out=cb[:, CB_EARLY:],
                              in_=cb_d.ap()[:, CB_EARLY:])
            for a0 in range(8, 32, 8):
                load3(xR, x0_d, a0, a0 + 8)

            # rotating eviction engine, weighted by per-op cost
            # (ACT ~672ns, DVE ~658ns, Pool ~925ns per [p,512])
            # GPSIMD cannot access PSUM on TRN2, so evictions rotate over
            # ACT/DVE only, weighted by per-op cost
            ectr = [0]
            cur_pat = ["AD"]

            def evict(dst, src):
                pat = cur_pat[0]
                r = pat[ectr[0] % len(pat)]
                ectr[0] += 1
                if r == "A":
                    nc.scalar.copy(dst, src)
                else:
                    nc.vector.tensor_copy(dst, src)

            def ds_stage(dstR, dstI, srcR, srcI, mA, mB, stat_view=None,
                         scatter=None, evict_pat=None, groups=None, lw=32):
                """Data-stationary FFT: per 128-block, out = blk^T @ [GR|GI].

                stat_view(src, b) returns the stationary AP for block b
                (defaults to contiguous 128-col slice). Output R/I halves land
                in psum as [.. | R(4*lw) | I(4*lw) ..] per block; eviction
                scatters them to dstR/dstI (contiguous unless scatter given).
                groups is a list of (block0, nblocks) contiguous runs.
                lw < 32 trims the moving operand to k2l < lw (W1 Hermitian).
                """
                cur_pat[0] = evict_pat or "AD"
                ectr[0] = 0
                if groups is None:
                    groups = [(4 * g, 4) for g in range(8)]
                # psum blocks sit on 256-col slots (so a trimmed 8*lw-wide
                # matmul output never crosses a 512-f32 psum bank boundary)
                for b0, nb in groups:
                    P = ps.tile([128, 256 * nb], f32, name="P", tag="ps")
                    for j in range(nb):
                        b = b0 + j
                        if stat_view is None:
                            sR = srcR[:, 128 * b:128 * (b + 1)]
                            sI = srcI[:, 128 * b:128 * (b + 1)] \
                                if srcI is not None else None
                        else:
                            sR = stat_view(srcR, b)
                            sI = stat_view(srcI, b) if srcI is not None else None
                        o = P[:, 256 * j:256 * j + 8 * lw]
                        if sI is None:
                            nc.tensor.matmul(o, sR, mA, start=True, stop=True)
                        else:
                            nc.tensor.matmul(o, sR, mA, start=True, stop=False)
                            nc.tensor.matmul(o, sI, mB, start=False, stop=True)
                    v = P.rearrange("p (b r) -> p b r", b=nb)
                    if scatter is None:
                        evict(dstR.rearrange("p (b k) -> p b k", b=32)
                              [:, b0:b0 + nb], v[:, :, 0:128])
                        evict(dstI.rearrange("p (b k) -> p b k", b=32)
                              [:, b0:b0 + nb], v[:, :, 128:256])
                    elif scatter == "S1":
                        # blocks b = d1, psum cols j = k2 = (k2h, k2l);
                        # scatter to free = k2l*128 + k2h*32 + d1 so the next
                        # stage's stationary is a contiguous 128-col block
                        # (hw requires single-free-dim stationary APs)
                        sR4 = v[:, :, 0:4 * lw].rearrange(
                            "p b (g l) -> p b g l", g=4)
                        sI4 = v[:, :, 4 * lw:8 * lw].rearrange(
                            "p b (g l) -> p b g l", g=4)
                        dv = [t.rearrange("p (l g d) -> p d g l", l=32, g=4)
                              [:, b0:b0 + nb, :, 0:lw]
                              for t in (dstR, dstI)]
                        evict(dv[0], sR4)
                        evict(dv[1], sI4)
                    else:
                        # S4: psum cols j=(g4,d32) per block b=k2l ->
                        # dst free = d1*128 + g*32 + k2l
                        sR4 = v[:, :, 0:128].rearrange(
                            "p b (g d) -> p g d b", g=4)
                        sI4 = v[:, :, 128:256].rearrange(
                            "p b (g d) -> p g d b", g=4)
                        dv = [t.rearrange("p (d g l) -> p g d l", d=32, g=4)
                              [:, :, :, b0:b0 + nb]
                              for t in (dstR, dstI)]
                        evict(dv[0], sR4)
                        evict(dv[1], sI4)

            def fstat_chunk(t, dst, src, mR, mI, mIn, mid=None, out_f32=False,
                            outdma=None, evict_pat=None, mid_swap=False):
                """One 1024-col chunk of: out_R = mR^T R + mIn^T I ;
                out_I = mI^T R + mR^T I.

                mid / out_f32 post-process per 512-col half as soon as its
                accumulation group closes, for finer pipelining into the
                next stage / output DMA.
                """
                def ptw_stage(hR, hI, cR, cI, o):
                    # stage pR/pI halves to bf16 SBUF as soon as each psum
                    # half closes (h0's copies hide under h1's matmuls);
                    # both on ACT -- DVE is the ptw-mult bottleneck
                    nc.scalar.copy(cR[:, o], hR[:, o])
                    nc.scalar.copy(cI[:, o], hI[:, o])

                def ptw(cRh, cIh, s, width, swap):
                    # fused pointwise: V = Z * W~ from the bf16 stagings (so
                    # DVE's SBUF-only bf16 ops run in 2x mode and Pool (no
                    # PSUM access) can participate). For reconstructed
                    # chunks the stored wI is -true-wI (swap=True):
                    #   normal: V_R = t1 - t2 ; V_I = t3 + t4
                    #   swap:   V_R = t1 + t2 ; V_I = t4 - t3
                    # with t1 = cR*wR, t2 = cI*wI, t3 = cR*wI, t4 = cI*wR.
                    mwR, mwI = mid
                    mu = mybir.AluOpType.mult
                    t1 = tp.tile([128, width], bf16, name="t1", tag="t1")
                    t2 = tp.tile([128, width], bf16, name="t2", tag="t2")
                    t3 = tp.tile([128, width], bf16, name="t3", tag="t1")
                    t4 = tp.tile([128, width], bf16, name="t4", tag="t2")
                    # in-order queues: emit each engine's dep-free mults
                    # first so it never stalls behind a cross-engine wait
                    nc.vector.tensor_tensor(t1, cRh, mwR[:, s], op=mu)
                    nc.vector.tensor_tensor(t3, cRh, mwI[:, s], op=mu)
                    nc.gpsimd.tensor_tensor(t2, cIh, mwI[:, s], op=mu)
                    nc.gpsimd.tensor_tensor(t4, cIh, mwR[:, s], op=mu)
                    sub = mybir.AluOpType.subtract
                    add = mybir.AluOpType.add
                    if not swap:
                        nc.gpsimd.tensor_tensor(dst[0][:, s], t1, t2, op=sub)
                        nc.vector.tensor_tensor(dst[1][:, s], t3, t4, op=add)
                    else:
                        nc.gpsimd.tensor_tensor(dst[0][:, s], t1, t2, op=add)
                        nc.vector.tensor_tensor(dst[1][:, s], t4, t3, op=sub)

                if evict_pat:
                    cur_pat[0] = evict_pat
                    ectr[0] = 0
                pR = ps.tile([128, 1024], f32, name="pR", tag="ps")
                pI = ps.tile([128, 1024], f32, name="pI", tag="ps")
                if mid is not None:
                    cR = tp.tile([128, 1024], bf16, name="cR", tag="tcR")
                    cI = tp.tile([128, 1024], bf16, name="cI", tag="tc")
                for h in range(2):
                    s = slice(1024 * t + 512 * h, 1024 * t + 512 * (h + 1))
                    o = slice(512 * h, 512 * (h + 1))
                    rhs = src[0][:, s]
                    rhsI = src[1][:, s]
                    nc.tensor.matmul(pR[:, o], M(mR), rhs,
                                     start=True, stop=False)
                    nc.tensor.matmul(pI[:, o], M(mI), rhs,
                                     start=True, stop=False)
                    nc.tensor.matmul(pR[:, o], M(mIn), rhsI,
                                     start=False, stop=True)
                    nc.tensor.matmul(pI[:, o], M(mR), rhsI,
                                     start=False, stop=True)
                    if out_f32:
                        # final stage: pinned engines, bf16 staging
                        nc.vector.tensor_copy(dst[0][:, s], pR[:, o])
                        nc.scalar.copy(dst[1][:, s], pI[:, o])
                        if outdma is not None:
                            outdma(2 * t + h)
                    elif mid is not None:
                        # per-half pointwise: S4's 4-block groups map 1:1 to
                        # ptw halves, so they unlock at half-chunk latency
                        ptw_stage(pR, pI, cR, cI, o)
                        ptw(cR[:, o], cI[:, o], s, 512, mid_swap)
                    else:
                        # W3: per-half evictions (h0's hide under h1's
                        # matmuls; the last op is half the size, freeing the
                        # psum ring for the next stage sooner)
                        evict(dst[0][:, s], pR[:, o])
                        evict(dst[1][:, s], pI[:, o])

            def fstat_stage(dst, src, mR, mI, mIn, mid=None, out_f32=False,
                            outdma=None):
                for t in range(4):
                    fstat_chunk(t, dst, src, mR, mI, mIn, mid=mid,
                                out_f32=out_f32, outdma=outdma)

            # ---------------- interleaved W / Z chains ----------------
            # W1: [d2|d1,d3] -(ds fwd, scaled, real)-> [d3|k2l,(k2h,d1)]
            # trimmed to k2l < LW (Hermitian: w real, so the k2l' >= LW half
            # of the spectrum is reconstructed after W3).
            # Emitted group-interleaved with S1 (z ds fwd d2, same output
            # layout) so S1's matmuls fill W1's eviction-paced PE gaps as
            # soon as x0/x1 chunks land.
            _mark(nc, "W1")
            for g in range(8):
                ds_stage(wR[1], wI[1], wL, None,
                         cb[:, CB_WS:CB_WS + 8 * LW], None, scatter="S1",
                         lw=LW, groups=[(4 * g, 4)])
                if g == 1:
                    _mark(nc, "S1")
                if g >= 1:
                    ds_stage(zR[1], zI[1], xR, xI, M2(CB_AF), M2(CB_BF),
                             scatter="S1", groups=[(4 * (g - 1), 4)])
            ds_stage(zR[1], zI[1], xR, xI, M2(CB_AF), M2(CB_BF),
                     scatter="S1", groups=[(28, 4)])
            # W2: ds fwd d3 -> [(k2h,d1)|k2l,k3], blocks k2l 0..16 only
            _mark(nc, "W2")
            ds_stage(wR[0], wI[0], wR[1], wI[1], M2(CB_AF), M2(CB_BF),
                     groups=[(0, 4), (4, 4), (8, 4), (12, 4), (16, 1)])
            # W3: Fstat BD fwd -> W~ [(k2h,k1)|k2l,k3], computed blocks only:
            # k2l 0..15 as two chunks, then block 16 alone with a negated-I
            # eviction so the pointwise sees one sign convention per chunk.
            # S2's first two groups are emitted ahead of W3: their matmuls
            # fill W3's eviction-paced PE holes (S1's output is long ready)
            ds_stage(zR[0], zI[0], zR[1], zI[1], M2(CB_AF), M2(CB_BF),
                     groups=[(0, 4)])
            _mark(nc, "W3")
            for t in range(2):
                fstat_chunk(t, (wR[1], wI[1]), (wR[0], wI[0]),
                            CB_BDR, CB_BDI, CB_BDIn, evict_pat="AD")
            sw = slice(2048, 2176)
            PwR = ps.tile([128, 128], f32, name="PwR", tag="ps")
            PwI = ps.tile([128, 128], f32, name="PwI", tag="ps")
            nc.tensor.matmul(PwR, M(CB_BDR), wR[0][:, sw],
                             start=True, stop=False)
            nc.tensor.matmul(PwI, M(CB_BDI), wR[0][:, sw],
                             start=True, stop=False)
            nc.tensor.matmul(PwR, M(CB_BDIn), wI[0][:, sw],
                             start=False, stop=True)
            nc.tensor.matmul(PwI, M(CB_BDR), wI[0][:, sw],
                             start=False, stop=True)
            nc.scalar.copy(wR[1][:, sw], PwR)
            nc.vector.tensor_scalar_mul(wI[1][:, sw], PwI, -1.0)

            # Hermitian reconstruction of W~ blocks k2l' 17..31 from block
            # 32-k2l':
            #   dst[32*k2h'+k1', 128*k2l'+k3']
            #     = src[32*(3-k2h') + (32-k1')%32, 128*(32-k2l') + (128-k3')%128]
            # The free-axis part (block + k3 reversal with wrap) runs on the
            # ALU (engines take negative free strides at full rate), giving
            # REVF[p, 128*b' + k3'] = V[p][14-b'][(128-k3')%128]. The
            # partition part must go through DMA (the only partition-permuting
            # path) with only whole-ROW reversal on the DRAM side -- each
            # descriptor stays a contiguous 3840B run. Row map: src row =
            # 128-p' for k1' != 0 (rows 127..1 for p' 1..127); the k1'=0
            # partitions (p' = 32*k2h', src row 96-32*k2h') come from a
            # 4-row fix-up read. Conjugation folds into ptw's swap variant.
            # R plane rides SP, I plane rides Pool; per-queue order gives
            # write -> read consistency on the scratch without extra syncs.
            _mark(nc, "RECON")
            for pi, (plane, wsc) in enumerate(((wR[1], wscR_d),
                                               (wI[1], wscI_d))):
                # free-axis reversal on DVE (4x bf16 SBUF mode), scratch
                # writes on the idle SP queue; gathers are gpsimd-only
                rev = sb.tile([128, 1920], bf16, name=f"rev{pi}")
                sv = plane[:, 128:2048].rearrange("p (b k) -> p b k", b=15)
                rv = rev.rearrange("p (b k) -> p b k", b=15)
                nc.gpsimd.tensor_copy(rv[:, :, 1:128], sv[:, 14::-1, 127:0:-1])
                nc.gpsimd.tensor_copy(rv[:, :, 0:1], sv[:, 14::-1, 0:1])
                nc.sync.dma_start(out=wsc.ap(), in_=rev)
                nc.gpsimd.indirect_dma_start(
                    out=plane[:, 17 * 128:4096], out_offset=None,
                    in_=wsc.ap(),
                    in_offset=bass.IndirectOffsetOnAxis(
                        ap=ridx[:, pi:pi + 1], axis=0))

            # S3: Fstat BD fwd + pointwise *W~ -> [(k2h,k1)|k2l,k3];
            # chunks 2,3 cover the reconstructed blocks (stored wI = -wI)
            # S2: ds fwd d3 -> [(k2h,d1)|k2l,k3]
            _mark(nc, "S2")
            ds_stage(zR[0], zI[0], zR[1], zI[1], M2(CB_AF), M2(CB_BF),
                     groups=[(4 * g, 4) for g in range(1, 8)])
            # S3 chunks and S4 groups emitted interleaved: S4 group g only
            # needs S3's pointwise for chunk g>>1, and the interleave keeps
            # the 4-deep psum ring from serializing S4's first allocation
            # behind S3's later chunks. S4 = ds inv k1 -> [k3|d1',k2].
            def s3(t):
                fstat_chunk(t, (zR[1], zI[1]), (zR[0], zI[0]),
                            CB_BDR, CB_BDI, CB_BDIn, mid=(wR[1], wI[1]),
                            mid_swap=(t >= 2))

            def s4(groups):
                ds_stage(zR[0], zI[0], zR[1], zI[1], M2(CB_ABI), M2(CB_BBI),
                         scatter="T3", evict_pat="AAD", groups=groups)

            _mark(nc, "S3")
            for t in range(4):
                s3(t)
            _mark(nc, "S4")
            s4([(0, 4), (4, 4), (8, 4), (12, 4), (16, 4),
                (20, 4), (24, 4), (28, 2), (30, 2)])
            # S5 + S6 interleaved: S6 chunk t only needs S5 groups 2t,2t+1
            # evicted, so emit [g0..g3, c0, g4g5, c1, g6g7, c2, c3] for a
            # stall-free pipeline into the output DMAs
            def s5(groups=None):
                ds_stage(zR[1], zI[1], zR[0], zI[0], M2(CB_AFI), M2(CB_BFI),
                         evict_pat="AD", groups=groups)

            # S6: Fstat inv k2 -> [d2'|d1',d3'] -> chunked DMA out
            # y0 on the SP queue, y1 on the Pool (SWDGE) queue, in parallel
            def outdma(th):
                a0, a1 = 4 * th, 4 * (th + 1)
                # last y1 chunk via ACT: its dependency (the I eviction) is
                # ACT's own last op, and HWDGE init beats Pool's SWDGE init
                y1_eng = nc.scalar if th == 7 else nc.gpsimd
                for eng, y_d, st in ((nc.sync, y0_d, yR),
                                     (y1_eng, y1_d, yI)):
                    eng.dma_start(
                        out=y_d.ap()[:, a0:a1],
                        in_=st.rearrange("p (a c) -> p a c", a=D1)[:, a0:a1])

            def s6(t):
                fstat_chunk(t, (yR, yI), (zR[1], zI[1]), CB_F2R, CB_F2In, CB_F2I,
                            out_f32=True, outdma=outdma)

            _mark(nc, "S5S6")
            s5()
            for t in range(4):
                s6(t)
            _mark(nc, "END")
    return nc


_CACHE = {}


def _get_program():
    if "nc" not in _CACHE:
        nc = _build_program()
        try:
            if not nc.is_finalized():
                nc.finalize()
        except AttributeError:
            nc.finalize()
        _CACHE["nc"] = nc
    return _CACHE["nc"]


def _run(x, w_real, **kw):
    from concourse.bass_utils import run_bass_kernel_spmd

    import ml_dtypes

    nc = _get_program()
    bf16 = ml_dtypes.bfloat16
    # pre-transpose to (D2, D1, D3) for contiguous per-partition DMA runs
    xt = np.ascontiguousarray(
        np.asarray(x, dtype=np.float32).transpose(0, 2, 1, 3)).astype(bf16)
    w = np.ascontiguousarray(
        np.asarray(w_real, dtype=np.float32).transpose(1, 0, 2)).astype(bf16)
    constsb = _consts_bf16_np()
    in_maps = []
    for c in range(NCORES):
        in_maps.append({
            "x0": xt[2 * c],
            "x1": xt[2 * c + 1],
            "w": w,
            "constsb": constsb,
            "ridx": _ridx_np(),
        })
    res = run_bass_kernel_spmd(nc, in_maps, core_ids=list(range(NCORES)), **kw)
    out = np.empty((B, D1, D2, D3), dtype=np.float32)
    for c in range(NCORES):
        out[2 * c] = res.results[c]["y0"].astype(np.float32).transpose(1, 0, 2)
        out[2 * c + 1] = (res.results[c]["y1"].astype(np.float32)
                          .transpose(1, 0, 2))
    return out, res


def kernel(x: np.ndarray, w_real: np.ndarray) -> np.ndarray:
    return _run(x, w_real)[0]


def kernel_traced(x: np.ndarray, w_real: np.ndarray):
    return _run(x, w_real, trace=True)

